# revision 1
# baseline (speedup 1.0000x reference)
"""Trainium2 Bass kernel for the DigitCap forward pass.

Math note: in the reference, C = softmax(sum(A, axis=-2, keepdims=True), axis=-2)
is a softmax over a size-1 axis, so C == 1.0 exactly for any finite input.
The whole attention gram matrix cancels and the computation reduces to

    S[b,m,d] = sum_n (1 + B_prior[m,0,n]) * sum_p W[m,n,d,p] * u[b,n,p]
    out      = squash(S) = (1 - exp(-|S|)) * S / (|S| + 1e-7)

Sharding: M=10 digit caps are covered by 5 cores holding 2 caps each
(uniform SPMD program; the remaining 3 cores run duplicate pairs whose
outputs are discarded). No collectives needed. Per-core the contraction
over (n, p) = 9216 runs as 9 n-chunks x 8 p accumulating fp32 matmuls
with W kept in its native (m, n, d, p) layout (512B-contiguous DMA).
"""

import os
import numpy as np

B = 16
N = 1152
DP = 8
M = 10
DD = 16
MS = 2           # m-slots per core
NCHUNK = N // 128
EPS = 1e-7

M_PAIRS = [(0, 1), (2, 3), (4, 5), (6, 7), (8, 9), (0, 1), (2, 3), (4, 5)]

_compiled = None


def _build():
    import concourse.bass as bass
    import concourse.tile as tile
    from concourse import bacc, mybir

    nc = bacc.Bacc("TRN2", target_bir_lowering=False, debug=False, num_devices=8)
    dt = mybir.dt.float32

    w_d = nc.dram_tensor("W_s", [MS, N, DD, DP], dt, kind="ExternalInput")
    u_d = nc.dram_tensor("uT", [N, DP, B], dt, kind="ExternalInput")
    bp_d = nc.dram_tensor("Bp", [MS, N], dt, kind="ExternalInput")
    out_d = nc.dram_tensor("out_s", [B, MS, DD], dt, kind="ExternalOutput")

    w_ap = w_d.ap()
    u_ap = u_d.ap()
    bp_ap = bp_d.ap()
    out_ap = out_d.ap()

    with tile.TileContext(nc) as tc:
        with (
            tc.tile_pool(name="wpool", bufs=3) as wpool,
            tc.tile_pool(name="upool", bufs=3) as upool,
            tc.tile_pool(name="cbpool", bufs=3) as cbpool,
            tc.tile_pool(name="psum", bufs=1, space="PSUM") as psum,
            tc.tile_pool(name="spool", bufs=1) as spool,
        ):
            acc = psum.tile([B, MS, DD], dt)
            for c in range(NCHUNK):
                wt = wpool.tile([128, MS, DD, DP], dt, tag="wt")
                nc.sync.dma_start(
                    wt[:],
                    w_ap[:, c * 128 : (c + 1) * 128].rearrange("m n d p -> n m d p"),
                )
                ut = upool.tile([128, DP, B], dt, tag="ut")
                nc.sync.dma_start(ut[:], u_ap[c * 128 : (c + 1) * 128])
                cbt = cbpool.tile([128, MS], dt, tag="cbt")
                nc.sync.dma_start(
                    cbt[:], bp_ap[:, c * 128 : (c + 1) * 128].rearrange("m n -> n m")
                )
                cb1 = cbpool.tile([128, MS], dt, tag="cb1")
                nc.vector.tensor_scalar_add(cb1[:], cbt[:], 1.0)
                # scale W chunk by (1 + B_prior) per (n, m-slot)
                for ms in range(MS):
                    nc.vector.tensor_scalar_mul(
                        wt[:, ms], wt[:, ms], cb1[:, ms : ms + 1]
                    )
                for p in range(DP):
                    nc.tensor.matmul(
                        acc[:],
                        ut[:, p],
                        wt[:, :, :, p],
                        start=(c == 0 and p == 0),
                        stop=(c == NCHUNK - 1 and p == DP - 1),
                    )

            # epilogue: squash over d per (b, m-slot)
            s = spool.tile([B, MS, DD], dt, tag="s")
            nc.vector.tensor_copy(s[:], acc[:])
            sq = spool.tile([B, MS, DD], dt, tag="sq")
            nc.scalar.square(sq[:], s[:])
            n2 = spool.tile([B, MS], dt, tag="n2")
            nc.vector.tensor_reduce(
                n2[:], sq[:], axis=mybir.AxisListType.X, op=mybir.AluOpType.add
            )
            nrm = spool.tile([B, MS], dt, tag="nrm")
            nc.scalar.sqrt(nrm[:], n2[:])
            e = spool.tile([B, MS], dt, tag="e")
            nc.scalar.activation(
                e[:], nrm[:], mybir.ActivationFunctionType.Exp, scale=-1.0
            )
            coef = spool.tile([B, MS], dt, tag="coef")
            nc.vector.tensor_scalar(
                coef[:], e[:], -1.0, 1.0, mybir.AluOpType.mult, mybir.AluOpType.add
            )
            neps = spool.tile([B, MS], dt, tag="neps")
            nc.vector.tensor_scalar_add(neps[:], nrm[:], EPS)
            rec = spool.tile([B, MS], dt, tag="rec")
            nc.vector.reciprocal(rec[:], neps[:])
            fac = spool.tile([B, MS], dt, tag="fac")
            nc.vector.tensor_mul(fac[:], coef[:], rec[:])
            o = spool.tile([B, MS, DD], dt, tag="o")
            for ms in range(MS):
                nc.vector.tensor_scalar_mul(o[:, ms], s[:, ms], fac[:, ms : ms + 1])
            nc.sync.dma_start(out_ap[:], o[:])

    nc.compile()
    return nc


def kernel(primary_caps: np.ndarray, W: np.ndarray, B_prior: np.ndarray) -> np.ndarray:
    from concourse.bass_utils import run_bass_kernel_spmd

    global _compiled
    if _compiled is None:
        _compiled = _build()
    nc = _compiled

    u = np.ascontiguousarray(primary_caps, dtype=np.float32)
    uT = np.ascontiguousarray(u.transpose(1, 2, 0))  # [N, DP, B]
    in_maps = []
    for pr in M_PAIRS:
        in_maps.append(
            {
                "W_s": np.ascontiguousarray(W[list(pr)], dtype=np.float32),
                "uT": uT,
                "Bp": np.ascontiguousarray(B_prior[list(pr), 0, :], dtype=np.float32),
            }
        )
    res = run_bass_kernel_spmd(nc, in_maps, list(range(8))).results
    out = np.empty((B, M, DD), dtype=np.float32)
    for i in range(5):
        out[:, 2 * i : 2 * i + 2, :] = res[i]["out_s"]
    return out


# revision 15
# speedup vs baseline: 1.2054x; 1.2054x over previous
"""Trainium2 Bass kernel for the DigitCap forward pass.

Math note: in the reference, C = softmax(sum(A, axis=-2, keepdims=True), axis=-2)
is a softmax over a size-1 axis, so C == 1.0 exactly for any finite input.
The whole attention gram matrix cancels and the computation reduces to

    S[b,m,d] = sum_n (1 + B_prior[m,0,n]) * sum_p W[m,n,d,p] * u[b,n,p]
    out      = squash(S) = (1 - exp(-|S|)) * S / (|S| + 1e-7)

Sharding: M=10 digit caps are covered by 5 cores holding 2 caps each
(uniform SPMD program; the remaining 3 cores run duplicate pairs whose
outputs are discarded). No collectives needed.

Compute per core: contraction over (n,p)=9216 as 9 n-chunks. Each chunk
is ONE wide matmul: lhsT = uT chunk [n=128, (p,b)=128] (stationary),
rhs = W chunk [n=128, (p',m',d)=256] (moving), accumulating into
PSUM[(p,b)=128, (p',m',d)=256]. The p'==p diagonal blocks are the wanted
partial sums; they are extracted and summed over p in the epilogue
(8x streamed compute waste, but the PE is fed 256-wide which it likes).
"""

import os
import numpy as np

B = 16
N = 1152
DP = 8
M = 10
DD = 16
MS = 2           # m-slots per core
NCHUNK = N // 128
EPS = 1e-7

M_PAIRS = [(0, 1), (2, 3), (4, 5), (6, 7), (8, 9), (0, 1), (2, 3), (4, 5)]

_compiled = None


def _build():
    import concourse.bass as bass
    import concourse.tile as tile
    from concourse import bacc, mybir

    mm_dt = os.environ.get("KERNEL_MM_DT", "f32")  # f32 | f32r | bf16
    n_wdma = int(os.environ.get("KERNEL_N_WDMA", "3"))  # W dma_start count

    nc = bacc.Bacc("TRN2", target_bir_lowering=False, debug=False, num_devices=8)
    f32 = mybir.dt.float32
    sb_dt = mybir.dt.bfloat16 if mm_dt == "bf16" else f32

    w_d = nc.dram_tensor("W_s", [MS, N, DD, DP], f32, kind="ExternalInput")
    u_d = nc.dram_tensor("uT", [N, DP, B], f32, kind="ExternalInput")
    bp_d = nc.dram_tensor("BpT", [N, MS], f32, kind="ExternalInput")
    sel_d = nc.dram_tensor("SEL", [128, DP, B], f32, kind="ExternalInput")
    out_d = nc.dram_tensor("out_s", [B, MS, DD], f32, kind="ExternalOutput")

    # source views, n-chunked to 128 partitions
    w_ap = w_d.ap().rearrange("m (c n) d p -> n c m (d p)", n=128)     # [128,9,2,128]
    u_ap = u_d.ap().rearrange("(c n) p b -> n c (p b)", n=128)         # [128,9,128]
    bp_ap = bp_d.ap().rearrange("(c n) m -> n c m", n=128)             # [128,9,2]
    out_ap = out_d.ap()

    with tile.TileContext(nc) as tc:
        with (
            tc.tile_pool(name="big", bufs=1) as big,
            tc.tile_pool(name="small", bufs=1) as small,
            tc.tile_pool(name="psum", bufs=1, space="PSUM") as psum,
        ):
            wt = big.tile([128, NCHUNK, MS, DD, DP], sb_dt, tag="wt")
            ut = big.tile([128, NCHUNK, DP, B], sb_dt, tag="ut")
            cbt = small.tile([128, NCHUNK, MS], f32, tag="cbt")
            dma_w = nc.gpsimd.dma_start if mm_dt == "bf16" else nc.sync.dma_start
            # W: split into n_wdma issues so chunk-group g's matmuls can
            # start while group g+1 is still in flight
            assert NCHUNK % n_wdma == 0
            gsz = NCHUNK // n_wdma
            for g in range(n_wdma):
                for ms in range(MS):
                    dma_w(
                        wt[:, g * gsz : (g + 1) * gsz, ms].rearrange(
                            "n c d p -> n c (d p)"
                        ),
                        w_ap[:, g * gsz : (g + 1) * gsz, ms],
                    )
            dma_u = nc.gpsimd.dma_start if mm_dt == "bf16" else nc.scalar.dma_start
            dma_u(ut[:].rearrange("n c p b -> n c (p b)"), u_ap)
            nc.scalar.dma_start(cbt[:], bp_ap)
            sel = big.tile([128, DP, B], f32, tag="sel")
            nc.scalar.dma_start(sel[:], sel_d.ap())

            cb1 = small.tile([128, NCHUNK, MS], f32, tag="cb1")
            nc.vector.tensor_scalar_add(cb1[:], cbt[:], 1.0)

            # scale W by (1 + B_prior), per (n, chunk, m-slot)
            for c in range(NCHUNK):
                for ms in range(MS):
                    eng = nc.vector if (c * MS + ms) % 3 else nc.scalar
                    if eng is nc.vector:
                        nc.vector.tensor_scalar_mul(
                            wt[:, c, ms], wt[:, c, ms], cb1[:, c, ms : ms + 1]
                        )
                    else:
                        nc.scalar.activation(
                            wt[:, c, ms],
                            wt[:, c, ms],
                            mybir.ActivationFunctionType.Copy,
                            scale=cb1[:, c, ms : ms + 1],
                        )

            ps = psum.tile([128, DP, MS, DD], f32, tag="ps")
            for c in range(NCHUNK):
                lhsT = ut[:, c].rearrange("n p b -> n (p b)")
                rhs = wt[:, c].rearrange("n m d p -> n p m d")
                if mm_dt == "f32r":
                    lhsT = lhsT.bitcast(mybir.dt.float32r)
                    rhs = rhs.bitcast(mybir.dt.float32r)
                nc.tensor.matmul(
                    ps[:],
                    lhsT,
                    rhs,
                    start=(c == 0),
                    stop=(c == NCHUNK - 1),
                )

            # diagonal extraction: S[b, m', d] = sum_p ps[16p+b, p, :, :].
            # DVE/walrus reject partition bases that aren't 32-aligned, so the
            # cross-partition gather runs on the PE: out2[b,:] accumulates
            # SEL[:, p].T @ ps_sb[:, p] over p, where SEL[q,p,b] = (q==16p+b).
            f32t = f32
            ps_sb = small.tile([128, DP, MS, DD], f32, tag="ps_sb")
            nc.vector.tensor_copy(ps_sb[:], ps[:])
            ps2 = psum.tile([B, MS, DD], f32, tag="ps2")
            for p in range(DP):
                nc.tensor.matmul(
                    ps2[:],
                    sel[:, p],
                    ps_sb[:, p],
                    start=(p == 0),
                    stop=(p == DP - 1),
                )
            s = small.tile([B, MS, DD], f32t, tag="s")
            nc.vector.tensor_copy(s[:], ps2[:])

            # squash over d per (b, m-slot)
            sq = small.tile([B, MS, DD], f32t, tag="sq")
            nc.vector.tensor_mul(sq[:], s[:], s[:])
            n2 = small.tile([B, MS], f32t, tag="n2")
            nc.vector.tensor_reduce(
                n2[:], sq[:], axis=mybir.AxisListType.X, op=mybir.AluOpType.add
            )
            nrm = small.tile([B, MS], f32t, tag="nrm")
            nc.scalar.sqrt(nrm[:], n2[:])
            e = small.tile([B, MS], f32t, tag="e")
            nc.scalar.activation(
                e[:], nrm[:], mybir.ActivationFunctionType.Exp, scale=-1.0
            )
            coef = small.tile([B, MS], f32t, tag="coef")
            nc.vector.tensor_scalar(
                coef[:], e[:], -1.0, 1.0, mybir.AluOpType.mult, mybir.AluOpType.add
            )
            neps = small.tile([B, MS], f32t, tag="neps")
            nc.vector.tensor_scalar_add(neps[:], nrm[:], EPS)
            rec = small.tile([B, MS], f32t, tag="rec")
            nc.vector.reciprocal(rec[:], neps[:])
            fac = small.tile([B, MS], f32t, tag="fac")
            nc.vector.tensor_mul(fac[:], coef[:], rec[:])
            o = small.tile([B, MS, DD], f32, tag="o")
            for ms in range(MS):
                nc.vector.tensor_scalar_mul(o[:, ms], s[:, ms], fac[:, ms : ms + 1])
            nc.sync.dma_start(out_ap[:], o[:])

    nc.compile()
    return nc


def make_in_maps(primary_caps: np.ndarray, W: np.ndarray, B_prior: np.ndarray):
    u = np.ascontiguousarray(primary_caps, dtype=np.float32)
    uT = np.ascontiguousarray(u.transpose(1, 2, 0))  # [N, DP, B]
    sel = np.zeros((128, DP, B), dtype=np.float32)
    for p in range(DP):
        for b in range(B):
            sel[16 * p + b, p, b] = 1.0
    in_maps = []
    for pr in M_PAIRS:
        in_maps.append(
            {
                "W_s": np.ascontiguousarray(W[list(pr)], dtype=np.float32),
                "uT": uT,
                "BpT": np.ascontiguousarray(
                    B_prior[list(pr), 0, :].T.astype(np.float32)
                ),
                "SEL": sel,
            }
        )
    return in_maps


def kernel(primary_caps: np.ndarray, W: np.ndarray, B_prior: np.ndarray) -> np.ndarray:
    from concourse.bass_utils import run_bass_kernel_spmd

    global _compiled
    if _compiled is None:
        _compiled = _build()
    nc = _compiled

    in_maps = make_in_maps(primary_caps, W, B_prior)
    res = run_bass_kernel_spmd(nc, in_maps, list(range(8))).results
    out = np.empty((B, M, DD), dtype=np.float32)
    for i in range(5):
        out[:, 2 * i : 2 * i + 2, :] = res[i]["out_s"]
    return out


# revision 20
# speedup vs baseline: 1.3188x; 1.0941x over previous
"""Trainium2 Bass kernel for the DigitCap forward pass.

Math note: in the reference, C = softmax(sum(A, axis=-2, keepdims=True), axis=-2)
is a softmax over a size-1 axis, so C == 1.0 exactly for any finite input.
The whole attention gram matrix cancels and the computation reduces to

    S[b,m,d] = sum_n (1 + B_prior[m,0,n]) * sum_p W[m,n,d,p] * u[b,n,p]
    out      = squash(S) = (1 - exp(-|S|)) * S / (|S| + 1e-7)

Sharding: M=10 digit caps are covered by 5 cores holding 2 caps each
(uniform SPMD program; the remaining 3 cores run duplicate pairs whose
outputs are discarded). No collectives needed.

Compute per core: contraction over (n,p)=9216 as 9 n-chunks. Each chunk
is ONE wide matmul: lhsT = uT chunk [n=128, (p,b)=128] (stationary),
rhs = W chunk [n=128, (p',m',d)=256] (moving), accumulating into
PSUM[(p,b)=128, (p',m',d)=256]. The p'==p diagonal blocks are the wanted
partial sums; they are extracted and summed over p in the epilogue
(8x streamed compute waste, but the PE is fed 256-wide which it likes).
"""

import os
import numpy as np

B = 16
N = 1152
DP = 8
M = 10
DD = 16
MS = 2           # m-slots per core
NCHUNK = N // 128
EPS = 1e-7

M_PAIRS = [(0, 1), (2, 3), (4, 5), (6, 7), (8, 9), (0, 1), (2, 3), (4, 5)]

_compiled = None


def _build():
    import concourse.bass as bass
    import concourse.tile as tile
    from concourse import bacc, mybir

    mm_dt = os.environ.get("KERNEL_MM_DT", "f32r")  # f32 | f32r | bf16
    n_wdma = int(os.environ.get("KERNEL_N_WDMA", "3"))  # W dma_start count

    nc = bacc.Bacc("TRN2", target_bir_lowering=False, debug=False, num_devices=8)
    f32 = mybir.dt.float32
    sb_dt = mybir.dt.bfloat16 if mm_dt == "bf16" else f32

    w_d = nc.dram_tensor("W_s", [MS, N, DD, DP], f32, kind="ExternalInput")
    u_d = nc.dram_tensor("uT", [N, DP, B], f32, kind="ExternalInput")
    bp_d = nc.dram_tensor("BpT", [N, MS], f32, kind="ExternalInput")
    sel_d = nc.dram_tensor("SEL", [128, DP, B], f32, kind="ExternalInput")
    out_d = nc.dram_tensor("out_s", [B, MS, DD], f32, kind="ExternalOutput")

    # source views, n-chunked to 128 partitions
    w_ap = w_d.ap().rearrange("m (c n) d p -> n c m (d p)", n=128)     # [128,9,2,128]
    u_ap = u_d.ap().rearrange("(c n) p b -> n c (p b)", n=128)         # [128,9,128]
    bp_ap = bp_d.ap().rearrange("(c n) m -> n c m", n=128)             # [128,9,2]
    out_ap = out_d.ap()

    with tile.TileContext(nc) as tc:
        with (
            tc.tile_pool(name="big", bufs=1) as big,
            tc.tile_pool(name="small", bufs=1) as small,
            tc.tile_pool(name="psum", bufs=1, space="PSUM") as psum,
        ):
            wt = big.tile([128, NCHUNK, MS, DD, DP], sb_dt, tag="wt")
            ut = big.tile([128, NCHUNK, DP, B], sb_dt, tag="ut")
            cbt = small.tile([128, NCHUNK, MS], f32, tag="cbt")
            sel = big.tile([128, DP, B], f32, tag="sel")
            dma_w = nc.gpsimd.dma_start if mm_dt == "bf16" else nc.sync.dma_start
            dma_u = nc.gpsimd.dma_start if mm_dt == "bf16" else nc.scalar.dma_start

            # tiny inputs first so cb1 and the first matmul aren't gated on
            # the big W transfers (HWDGE completion is FIFO per queue lane)
            nc.sync.dma_start(cbt[:], bp_ap)
            dma_u(ut[:].rearrange("n c p b -> n c (p b)"), u_ap)
            nc.scalar.dma_start(sel[:], sel_d.ap())

            # W: split into n_wdma issues so chunk-group g's matmuls can
            # start while group g+1 is still in flight
            assert NCHUNK % n_wdma == 0
            gsz = NCHUNK // n_wdma
            for g in range(n_wdma):
                for ms in range(MS):
                    if mm_dt == "bf16":
                        eng_dma = nc.gpsimd.dma_start
                    else:
                        eng_dma = nc.sync.dma_start if ms == 0 else nc.scalar.dma_start
                    eng_dma(
                        wt[:, g * gsz : (g + 1) * gsz, ms].rearrange(
                            "n c d p -> n c (d p)"
                        ),
                        w_ap[:, g * gsz : (g + 1) * gsz, ms],
                    )

            cb1 = small.tile([128, NCHUNK, MS], f32, tag="cb1")
            nc.vector.tensor_scalar_add(cb1[:], cbt[:], 1.0)

            # hoist ACT table loads (Sqrt/Exp) so they overlap the DMA phase
            # instead of stalling the epilogue chain
            warm = small.tile([B, MS], f32, tag="warm")
            nc.scalar.activation(
                warm[:], cb1[0:B, 0], mybir.ActivationFunctionType.Sqrt
            )
            nc.scalar.activation(
                warm[:], warm[:], mybir.ActivationFunctionType.Exp, scale=-1.0
            )

            if mm_dt == "f32r":
                f32r = mybir.dt.float32r
                wt_s = big.tile([128, NCHUNK, MS, DD, DP], f32r, tag="wt_s")
                ut_mm = big.tile([128, NCHUNK, DP, B], f32r, tag="ut_mm")
                nc.vector.tensor_copy(ut_mm[:], ut[:])  # rounds to f32r
            else:
                wt_s = wt
                ut_mm = ut

            ps = psum.tile([128, DP, MS, DD], f32, tag="ps")
            for c in range(NCHUNK):
                # scale W by (1 + B_prior), per (n, chunk, m-slot); for f32r
                # this op also performs the required rounding on its output
                for ms in range(MS):
                    nc.vector.tensor_scalar_mul(
                        wt_s[:, c, ms], wt[:, c, ms], cb1[:, c, ms : ms + 1]
                    )
                nc.tensor.matmul(
                    ps[:],
                    ut_mm[:, c].rearrange("n p b -> n (p b)"),
                    wt_s[:, c].rearrange("n m d p -> n p m d"),
                    start=(c == 0),
                    stop=(c == NCHUNK - 1),
                )

            # diagonal extraction: S[b, m', d] = sum_p ps[16p+b, p, :, :].
            # DVE/walrus reject partition bases that aren't 32-aligned, so the
            # cross-partition gather runs on the PE: out2[b,:] accumulates
            # SEL[:, p].T @ ps_sb[:, p] over p, where SEL[q,p,b] = (q==16p+b).
            f32t = f32
            ps_sb = small.tile([128, DP, MS, DD], f32, tag="ps_sb")
            nc.vector.tensor_copy(ps_sb[:], ps[:])
            ps2 = psum.tile([B, MS, DD], f32, tag="ps2")
            for p in range(DP):
                nc.tensor.matmul(
                    ps2[:],
                    sel[:, p],
                    ps_sb[:, p],
                    start=(p == 0),
                    stop=(p == DP - 1),
                )
            s = small.tile([B, MS, DD], f32t, tag="s")
            nc.vector.tensor_copy(s[:], ps2[:])

            # squash over d per (b, m-slot): Square with accum_out fuses the
            # square and the d-reduction into one ACT op per m-slot
            sq = small.tile([B, MS, DD], f32t, tag="sq")
            n2 = small.tile([B, MS], f32t, tag="n2")
            for ms in range(MS):
                nc.scalar.activation(
                    sq[:, ms],
                    s[:, ms],
                    mybir.ActivationFunctionType.Square,
                    accum_out=n2[:, ms : ms + 1],
                )
            nrm = small.tile([B, MS], f32t, tag="nrm")
            nc.scalar.sqrt(nrm[:], n2[:])
            e = small.tile([B, MS], f32t, tag="e")
            nc.scalar.activation(
                e[:], nrm[:], mybir.ActivationFunctionType.Exp, scale=-1.0
            )
            coef = small.tile([B, MS], f32t, tag="coef")
            nc.vector.tensor_scalar(
                coef[:], e[:], -1.0, 1.0, mybir.AluOpType.mult, mybir.AluOpType.add
            )
            neps = small.tile([B, MS], f32t, tag="neps")
            nc.vector.tensor_scalar_add(neps[:], nrm[:], EPS)
            rec = small.tile([B, MS], f32t, tag="rec")
            nc.vector.reciprocal(rec[:], neps[:])
            fac = small.tile([B, MS], f32t, tag="fac")
            nc.vector.tensor_mul(fac[:], coef[:], rec[:])
            o = small.tile([B, MS, DD], f32, tag="o")
            for ms in range(MS):
                nc.vector.tensor_scalar_mul(o[:, ms], s[:, ms], fac[:, ms : ms + 1])
            nc.sync.dma_start(out_ap[:], o[:])

    nc.compile()
    return nc


def make_in_maps(primary_caps: np.ndarray, W: np.ndarray, B_prior: np.ndarray):
    u = np.ascontiguousarray(primary_caps, dtype=np.float32)
    uT = np.ascontiguousarray(u.transpose(1, 2, 0))  # [N, DP, B]
    sel = np.zeros((128, DP, B), dtype=np.float32)
    for p in range(DP):
        for b in range(B):
            sel[16 * p + b, p, b] = 1.0
    in_maps = []
    for pr in M_PAIRS:
        in_maps.append(
            {
                "W_s": np.ascontiguousarray(W[list(pr)], dtype=np.float32),
                "uT": uT,
                "BpT": np.ascontiguousarray(
                    B_prior[list(pr), 0, :].T.astype(np.float32)
                ),
                "SEL": sel,
            }
        )
    return in_maps


def kernel(primary_caps: np.ndarray, W: np.ndarray, B_prior: np.ndarray) -> np.ndarray:
    from concourse.bass_utils import run_bass_kernel_spmd

    global _compiled
    if _compiled is None:
        _compiled = _build()
    nc = _compiled

    in_maps = make_in_maps(primary_caps, W, B_prior)
    res = run_bass_kernel_spmd(nc, in_maps, list(range(8))).results
    out = np.empty((B, M, DD), dtype=np.float32)
    for i in range(5):
        out[:, 2 * i : 2 * i + 2, :] = res[i]["out_s"]
    return out


# revision 23
# speedup vs baseline: 1.4606x; 1.1076x over previous
"""Trainium2 Bass kernel for the DigitCap forward pass.

Math note: in the reference, C = softmax(sum(A, axis=-2, keepdims=True), axis=-2)
is a softmax over a size-1 axis, so C == 1.0 exactly for any finite input.
The whole attention gram matrix cancels and the computation reduces to

    S[b,m,d] = sum_n (1 + B_prior[m,0,n]) * sum_p W[m,n,d,p] * u[b,n,p]
    out      = squash(S) = (1 - exp(-|S|)) * S / (|S| + 1e-7)

Sharding: M=10 digit caps are covered by 5 cores holding 2 caps each
(uniform SPMD program; the remaining 3 cores run duplicate pairs whose
outputs are discarded). No collectives needed.

Compute per core: contraction over (n,p)=9216 as 9 n-chunks. Each chunk
is ONE wide matmul: lhsT = uT chunk [n=128, (p,b)=128] (stationary),
rhs = W chunk [n=128, (p',m',d)=256] (moving), accumulating into
PSUM[(p,b)=128, (p',m',d)=256]. The p'==p diagonal blocks are the wanted
partial sums; they are extracted and summed over p in the epilogue
(8x streamed compute waste, but the PE is fed 256-wide which it likes).
"""

import os
import numpy as np

B = 16
N = 1152
DP = 8
M = 10
DD = 16
MS = 2           # m-slots per core
NCHUNK = N // 128
EPS = 1e-7

M_PAIRS = [(0, 1), (2, 3), (4, 5), (6, 7), (8, 9), (0, 1), (2, 3), (4, 5)]

_compiled = None


def _build():
    import concourse.bass as bass
    import concourse.tile as tile
    from concourse import bacc, mybir

    mm_dt = os.environ.get("KERNEL_MM_DT", "f32r")  # f32 | f32r | bf16
    n_wdma = int(os.environ.get("KERNEL_N_WDMA", "3"))  # W dma_start count

    nc = bacc.Bacc("TRN2", target_bir_lowering=False, debug=False, num_devices=8)
    f32 = mybir.dt.float32
    sb_dt = mybir.dt.bfloat16 if mm_dt == "bf16" else f32

    w_d = nc.dram_tensor("W_s", [MS, N, DD, DP], f32, kind="ExternalInput")
    u_d = nc.dram_tensor("uT", [N, DP, B], f32, kind="ExternalInput")
    bp_d = nc.dram_tensor("BpT", [N, MS], f32, kind="ExternalInput")
    sel_d = nc.dram_tensor("SEL", [128, DP, B], f32, kind="ExternalInput")
    out_d = nc.dram_tensor("out_s", [B, MS, DD], f32, kind="ExternalOutput")

    # source views, n-chunked to 128 partitions
    w_ap = w_d.ap().rearrange("m (c n) d p -> n c m (d p)", n=128)     # [128,9,2,128]
    u_ap = u_d.ap().rearrange("(c n) p b -> n c (p b)", n=128)         # [128,9,128]
    bp_ap = bp_d.ap().rearrange("(c n) m -> n c m", n=128)             # [128,9,2]
    out_ap = out_d.ap()

    with tile.TileContext(nc) as tc:
        with (
            tc.tile_pool(name="big", bufs=1) as big,
            tc.tile_pool(name="small", bufs=1) as small,
            tc.tile_pool(name="psum", bufs=1, space="PSUM") as psum,
        ):
            wt = big.tile([128, NCHUNK, MS, DD, DP], sb_dt, tag="wt")
            ut = big.tile([128, NCHUNK, DP, B], sb_dt, tag="ut")
            cbt = small.tile([128, NCHUNK, MS], f32, tag="cbt")
            sel = big.tile([128, DP, B], f32, tag="sel")
            dma_w = nc.gpsimd.dma_start if mm_dt == "bf16" else nc.sync.dma_start
            dma_u = nc.gpsimd.dma_start if mm_dt == "bf16" else nc.scalar.dma_start

            # tiny inputs first so cb1 and the first matmul aren't gated on
            # the big W transfers (HWDGE completion is FIFO per queue lane)
            nc.sync.dma_start(cbt[:], bp_ap)
            dma_u(ut[:].rearrange("n c p b -> n c (p b)"), u_ap)
            nc.scalar.dma_start(sel[:], sel_d.ap())

            # W: split into n_wdma issues so chunk-group g's matmuls can
            # start while group g+1 is still in flight
            assert NCHUNK % n_wdma == 0
            gsz = NCHUNK // n_wdma
            for g in range(n_wdma):
                for ms in range(MS):
                    if mm_dt == "bf16":
                        eng_dma = nc.gpsimd.dma_start
                    else:
                        eng_dma = nc.sync.dma_start if ms == 0 else nc.scalar.dma_start
                    eng_dma(
                        wt[:, g * gsz : (g + 1) * gsz, ms].rearrange(
                            "n c d p -> n c (d p)"
                        ),
                        w_ap[:, g * gsz : (g + 1) * gsz, ms],
                    )

            cb1 = small.tile([128, NCHUNK, MS], f32, tag="cb1")
            nc.vector.tensor_scalar_add(cb1[:], cbt[:], 1.0)

            # hoist ACT table loads (Sqrt/Exp, the only two ACT funcs used) so
            # they overlap the DMA phase instead of stalling the epilogue chain
            warm = small.tile([B, MS], f32, tag="warm")
            nc.scalar.activation(
                warm[:], cb1[0:B, 0], mybir.ActivationFunctionType.Exp, scale=-1.0
            )
            nc.scalar.activation(
                warm[:], warm[:], mybir.ActivationFunctionType.Sqrt
            )

            if mm_dt == "f32r":
                f32r = mybir.dt.float32r
                wt_s = big.tile([128, NCHUNK, MS, DD, DP], f32r, tag="wt_s")
                ut_mm = big.tile([128, NCHUNK, DP, B], f32r, tag="ut_mm")
                nc.vector.tensor_copy(ut_mm[:], ut[:])  # rounds to f32r
            else:
                wt_s = wt
                ut_mm = ut

            ps = psum.tile([128, DP, MS, DD], f32, tag="ps")
            for c in range(NCHUNK):
                # scale W by (1 + B_prior), per (n, chunk, m-slot); for f32r
                # this op also performs the required rounding on its output
                for ms in range(MS):
                    nc.vector.tensor_scalar_mul(
                        wt_s[:, c, ms], wt[:, c, ms], cb1[:, c, ms : ms + 1]
                    )
                nc.tensor.matmul(
                    ps[:],
                    ut_mm[:, c].rearrange("n p b -> n (p b)"),
                    wt_s[:, c].rearrange("n m d p -> n p m d"),
                    start=(c == 0),
                    stop=(c == NCHUNK - 1),
                )

            # diagonal extraction: S[b, m', d] = sum_p ps[16p+b, p, :, :].
            # DVE/walrus reject partition bases that aren't 32-aligned, so the
            # cross-partition gather runs on the PE: out2[b,:] accumulates
            # SEL[:, p].T @ ps_sb[:, p] over p, where SEL[q,p,b] = (q==16p+b).
            f32t = f32
            ps_sb = small.tile([128, DP, MS, DD], f32, tag="ps_sb")
            nc.vector.tensor_copy(ps_sb[:], ps[:])
            ps2 = psum.tile([B, MS, DD], f32, tag="ps2")
            for p in range(DP):
                nc.tensor.matmul(
                    ps2[:],
                    sel[:, p],
                    ps_sb[:, p],
                    start=(p == 0),
                    stop=(p == DP - 1),
                )
            s = small.tile([B, MS, DD], f32t, tag="s")
            nc.vector.tensor_copy(s[:], ps2[:])

            # squash over d per (b, m-slot)
            sq = small.tile([B, MS, DD], f32t, tag="sq")
            nc.vector.tensor_mul(sq[:], s[:], s[:])
            n2 = small.tile([B, MS], f32t, tag="n2")
            nc.vector.tensor_reduce(
                n2[:], sq[:], axis=mybir.AxisListType.X, op=mybir.AluOpType.add
            )
            nrm = small.tile([B, MS], f32t, tag="nrm")
            nc.scalar.sqrt(nrm[:], n2[:])
            e = small.tile([B, MS], f32t, tag="e")
            nc.scalar.activation(
                e[:], nrm[:], mybir.ActivationFunctionType.Exp, scale=-1.0
            )
            coef = small.tile([B, MS], f32t, tag="coef")
            nc.vector.tensor_scalar(
                coef[:], e[:], -1.0, 1.0, mybir.AluOpType.mult, mybir.AluOpType.add
            )
            neps = small.tile([B, MS], f32t, tag="neps")
            nc.vector.tensor_scalar_add(neps[:], nrm[:], EPS)
            rec = small.tile([B, MS], f32t, tag="rec")
            nc.vector.reciprocal(rec[:], neps[:])
            fac = small.tile([B, MS], f32t, tag="fac")
            nc.vector.tensor_mul(fac[:], coef[:], rec[:])
            o = small.tile([B, MS, DD], f32, tag="o")
            for ms in range(MS):
                nc.vector.tensor_scalar_mul(o[:, ms], s[:, ms], fac[:, ms : ms + 1])
            nc.sync.dma_start(out_ap[:], o[:])

    nc.compile()
    return nc


def make_in_maps(primary_caps: np.ndarray, W: np.ndarray, B_prior: np.ndarray):
    u = np.ascontiguousarray(primary_caps, dtype=np.float32)
    uT = np.ascontiguousarray(u.transpose(1, 2, 0))  # [N, DP, B]
    sel = np.zeros((128, DP, B), dtype=np.float32)
    for p in range(DP):
        for b in range(B):
            sel[16 * p + b, p, b] = 1.0
    in_maps = []
    for pr in M_PAIRS:
        in_maps.append(
            {
                "W_s": np.ascontiguousarray(W[list(pr)], dtype=np.float32),
                "uT": uT,
                "BpT": np.ascontiguousarray(
                    B_prior[list(pr), 0, :].T.astype(np.float32)
                ),
                "SEL": sel,
            }
        )
    return in_maps


def kernel(primary_caps: np.ndarray, W: np.ndarray, B_prior: np.ndarray) -> np.ndarray:
    from concourse.bass_utils import run_bass_kernel_spmd

    global _compiled
    if _compiled is None:
        _compiled = _build()
    nc = _compiled

    in_maps = make_in_maps(primary_caps, W, B_prior)
    res = run_bass_kernel_spmd(nc, in_maps, list(range(8))).results
    out = np.empty((B, M, DD), dtype=np.float32)
    for i in range(5):
        out[:, 2 * i : 2 * i + 2, :] = res[i]["out_s"]
    return out
